# revision 28
# baseline (speedup 1.0000x reference)
"""MoE layer (8 experts, top-2) as an expert-parallel Trainium2 Bass kernel.

Strategy:
  - Host: gating matmul (tiny), top-2 routing, gather tokens per expert with
    a fixed device capacity C=2048 (= mean load); overflow pairs (a few
    hundred) are computed on host in fp32 (standard capacity-factor overflow).
  - Device (8 NeuronCores, SPMD, one expert per core): fused FFN over token
    chunks of 512 — for each chunk: h = relu(x @ W1 + b1) stays in SBUF,
    then y = (h @ W2) * combine_weight — bf16 with fp32 PSUM accumulation.
    No DRAM round-trip for h; W1 is streamed per chunk, W2 is resident.
  - Host: scatter-add the two expert contributions per token, add b2 term.

Layouts (device side, per core; p = SBUF partition 0..127):
  xT : [128, 4*8*512]  bf16  xT[p, ci*4096 + ko*512 + c] = x_tok[ci*512+c, ko*128+p]
  w1 : [128, 8*4*8*128] bf16 w1[p, ((q*4 + ftl)*8 + ko)*128 + f]
                                  = W1[ko*128+p, (q*4 + ftl)*128 + f]
  w2 : [128, 32*1024]  bf16  w2[p, ko2*1024 + d]         = W2[ko2*128+p, d]
  b1 : [128, 32]       f32   b1[p, ft]                   = b1_vec[ft*128+p]
  cw : [128, 16]       f32   cw[p, o]                    = combine_weight[o*128+p]
  y  : [128, 16*1024]  f32 (out)  y[p, o*1024+d] = y_tok[o*128+p, d]
"""

import os

import numpy as np
import ml_dtypes

D_MODEL = 1024
D_FF = 4096
N_EXPERTS = 8
TOP_K = 2
B, S = 4, 2048
T = B * S
P = 128
KO1 = D_MODEL // P   # 8  k-subtiles for matmul1
KO2 = D_FF // P      # 32 k-subtiles for matmul2
N_CORES = 8

C = 2048             # device capacity per expert (mean load)
TB = 512             # token chunk
NCHUNK = C // TB     # 4
OUTERS = C // P      # 16
FT = D_FF // P       # 32 f-tiles
FE = 512             # W1 slab width (f)
FT_E = FE // P       # 4 f-tiles per slab
NSLAB = D_FF // FE   # 8 slabs

BF16 = ml_dtypes.bfloat16

_NC_CACHE = {}
LAST_RESULTS = None  # BassKernelResults of the most recent run (for test.py)
LAST_IN_MAPS = None  # per-core input maps of the most recent run
LAST_C = None


def _build_nc(c_unused=None, reps=1):
    import concourse.bass as bass  # noqa: F401
    import concourse.tile as tile
    from concourse import bacc, mybir
    from contextlib import ExitStack

    nc = bacc.Bacc("TRN2", target_bir_lowering=False, debug=False,
                   num_devices=N_CORES)

    xT = nc.dram_tensor("xT", [P, NCHUNK * KO1 * TB], mybir.dt.bfloat16,
                        kind="ExternalInput")
    w1 = nc.dram_tensor("w1", [P, NSLAB * KO1 * FE], mybir.dt.bfloat16,
                        kind="ExternalInput")
    w2 = nc.dram_tensor("w2", [P, KO2 * D_MODEL], mybir.dt.bfloat16,
                        kind="ExternalInput")
    b1 = nc.dram_tensor("b1", [P, FT], mybir.dt.float32,
                        kind="ExternalInput")
    cw = nc.dram_tensor("cw", [P, OUTERS], mybir.dt.float32,
                        kind="ExternalInput")
    y = nc.dram_tensor("y", [P, OUTERS * D_MODEL], mybir.dt.float32,
                       kind="ExternalOutput")

    xT_ap = xT.ap().rearrange("p (ci ko c) -> p ci ko c", ci=NCHUNK, ko=KO1)
    w1_ap = w1.ap().rearrange("p (q ftl ko f) -> p q ftl ko f",
                              q=NSLAB, ftl=FT_E, ko=KO1)
    w2_ap = w2.ap().rearrange("p (ko d) -> p ko d", ko=KO2)
    y_ap = y.ap()

    with tile.TileContext(nc) as tc, ExitStack() as ctx:
        wpool = ctx.enter_context(tc.tile_pool(name="wpool", bufs=1))
        w1pool = ctx.enter_context(tc.tile_pool(name="w1pool", bufs=3))
        xpool = ctx.enter_context(tc.tile_pool(name="xpool", bufs=2))
        hpool = ctx.enter_context(tc.tile_pool(name="hpool", bufs=2))
        ypool = ctx.enter_context(tc.tile_pool(name="ypool", bufs=3))
        ps1 = ctx.enter_context(tc.tile_pool(name="ps1", bufs=4, space="PSUM"))
        ps2 = ctx.enter_context(tc.tile_pool(name="ps2", bufs=4, space="PSUM"))

        # (reps>1 repeats the whole body back-to-back; timing-only)
        for _rep in range(reps):
            W2s = wpool.tile([P, KO2, D_MODEL], mybir.dt.bfloat16, tag="W2s",
                             name="W2s")
            b1s = wpool.tile([P, FT], mybir.dt.float32, tag="b1s", name="b1s")
            cws = wpool.tile([P, OUTERS], mybir.dt.float32, tag="cws",
                             name="cws")

            nc.scalar.dma_start(b1s[:], b1.ap())
            nc.scalar.dma_start(cws[:], cw.ap())

            # PE warm-up: the HAM clock gate holds the PE at 1.2 GHz until
            # it has seen ~3.4us of sustained activity. The first real
            # matmul can't start until the x/W1 DMAs land (~14us), so spend
            # the idle window on dummy matmuls over a zeroed tile; the real
            # stream then starts at the full 2.4 GHz.
            scratch = wpool.tile([P, 512], mybir.dt.bfloat16, tag="warm",
                                 name="scratch")
            nc.any.memset(scratch[:], 0)
            warm = ps2.tile([P, 512], mybir.dt.float32, tag="ps2",
                            name="warm")
            for _ in range(23):
                nc.tensor.matmul(warm[:], scratch[:, 0:P], scratch[:],
                                 start=True, stop=True)

            for ci in range(NCHUNK):
                xc = xpool.tile([P, KO1, TB], mybir.dt.bfloat16, tag="x",
                                name="xc")
                nc.sync.dma_start(xc[:], xT_ap[:, ci])
                hsb = hpool.tile([P, FT, TB], mybir.dt.bfloat16, tag="h",
                                 name="hsb")

                # mm1: h^T[f, tok] = relu(W1^T x^T + b1), chunk resident
                # One DMA per W1 slab: DMA issue costs ~0.7us of engine time
                # each and queue slots are scarce, so fewer+larger wins.
                for q in range(NSLAB):
                    w1s = w1pool.tile([P, FT_E, KO1, P], mybir.dt.bfloat16,
                                      tag="w1s", name="w1s")
                    nc.sync.dma_start(w1s[:], w1_ap[:, q])
                    if ci == 0 and q == 1:
                        # W2 bulk load on the scalar queue, in ko-order
                        # pieces: early enough to be resident when mm2(c0)
                        # starts, fine-grained so mm2 can chase the tail.
                        for j in range(0, KO2, 4):
                            nc.scalar.dma_start(W2s[:, j:j + 4, :],
                                                w2_ap[:, j:j + 4, :])
                    for ftl in range(FT_E):
                        ft = q * FT_E + ftl
                        pt = ps1.tile([P, TB], mybir.dt.float32, tag="ps1",
                                      name="pt")
                        for ko in range(KO1):
                            nc.tensor.matmul(
                                pt[:],
                                w1s[:, ftl, ko, :],
                                xc[:, ko, :],
                                start=(ko == 0),
                                stop=(ko == KO1 - 1),
                            )
                        nc.scalar.activation(
                            hsb[:, ft, :], pt[:],
                            mybir.ActivationFunctionType.Relu,
                            bias=b1s[:, ft:ft + 1],
                        )

                # mm2: y[tok, d] = (h @ W2) * cw
                for ms in range(TB // P):
                    last = (ci == NCHUNK - 1 and ms == TB // P - 1)
                    pa = ps2.tile([P, 512], mybir.dt.float32, tag="ps2",
                                  name="pa")
                    pb = ps2.tile([P, 512], mybir.dt.float32, tag="ps2",
                                  name="pb")
                    outer = ci * (TB // P) + ms
                    yt = ypool.tile([P, D_MODEL], mybir.dt.float32, tag="y",
                                    name="yt")
                    if last:
                        # Drain overlap: finish pa's half first and store it
                        # on the idle sync engine under pb's matmuls; pb is
                        # further split into two 256-wide psum groups so the
                        # second-to-last store also hides under matmuls and
                        # only a 256-wide store trails the final matmul.
                        for ko in range(KO2):
                            nc.tensor.matmul(
                                pa[:], hsb[:, ko, ms * P:(ms + 1) * P],
                                W2s[:, ko, 0:512],
                                start=(ko == 0), stop=(ko == KO2 - 1))
                        nc.scalar.mul(yt[:, 0:512], pa[:],
                                      cws[:, outer:outer + 1])
                        nc.sync.dma_start(
                            y_ap[:, outer * D_MODEL:outer * D_MODEL + 512],
                            yt[:, 0:512])
                        for d0 in (512, 768):
                            for ko in range(KO2):
                                nc.tensor.matmul(
                                    pb[:, 0:256],
                                    hsb[:, ko, ms * P:(ms + 1) * P],
                                    W2s[:, ko, d0:d0 + 256],
                                    start=(ko == 0), stop=(ko == KO2 - 1))
                            nc.scalar.mul(yt[:, d0:d0 + 256], pb[:, 0:256],
                                          cws[:, outer:outer + 1])
                            eng = nc.sync if d0 == 512 else nc.scalar
                            eng.dma_start(
                                y_ap[:, outer * D_MODEL + d0:
                                     outer * D_MODEL + d0 + 256],
                                yt[:, d0:d0 + 256])
                        continue
                    for ko in range(KO2):
                        lhsT = hsb[:, ko, ms * P:(ms + 1) * P]
                        nc.tensor.matmul(pa[:], lhsT, W2s[:, ko, 0:512],
                                         start=(ko == 0), stop=(ko == KO2 - 1))
                        nc.tensor.matmul(pb[:], lhsT, W2s[:, ko, 512:1024],
                                         start=(ko == 0), stop=(ko == KO2 - 1))
                    nc.scalar.mul(yt[:, 0:512], pa[:], cws[:, outer:outer + 1])
                    nc.scalar.mul(yt[:, 512:1024], pb[:],
                                  cws[:, outer:outer + 1])
                    nc.scalar.dma_start(
                        y_ap[:, outer * D_MODEL:(outer + 1) * D_MODEL],
                        yt[:],
                    )

    nc.compile()
    return nc


def _route(x_flat, Wg, bg):
    logits = x_flat.astype(np.float32) @ Wg.astype(np.float32) + bg
    idx = np.argsort(-logits, axis=1, kind="stable")[:, :TOP_K]
    gates = np.take_along_axis(logits, idx, axis=1)  # [T, 2] descending
    e1 = np.exp(gates[:, 1] - gates[:, 0])
    denom = 1.0 + e1
    w = np.stack([1.0 / denom, e1 / denom], axis=1).astype(np.float32)
    return idx.astype(np.int32), w


def kernel(x, Wg, bg, W1, b1, W2, b2):
    global LAST_RESULTS
    x = np.asarray(x, dtype=np.float32)
    Wg = np.asarray(Wg, dtype=np.float32)
    bg = np.asarray(bg, dtype=np.float32)
    W1 = np.asarray(W1, dtype=np.float32)
    b1 = np.asarray(b1, dtype=np.float32)
    W2 = np.asarray(W2, dtype=np.float32)
    b2 = np.asarray(b2, dtype=np.float32)

    x_flat = x.reshape(T, D_MODEL)
    idx, w = _route(x_flat, Wg, bg)

    # Per-expert token lists capped at C; overflow pairs spill to host.
    # slot[t, k] = position in expert block, or -1 if spilled.
    tok_lists = []
    spill_lists = []
    slot = np.full((T, TOP_K), -1, dtype=np.int64)
    for e in range(N_EXPERTS):
        mask = (idx[:, 0] == e) | (idx[:, 1] == e)
        tok = np.nonzero(mask)[0]
        kept, spilled = tok[:C], tok[C:]
        tok_lists.append(kept)
        spill_lists.append(spilled)
        which = (idx[kept, 1] == e).astype(np.int64)  # k slot of expert e
        slot[kept, which] = np.arange(len(kept))

    if "nc" not in _NC_CACHE:
        _NC_CACHE["nc"] = _build_nc()
    nc = _NC_CACHE["nc"]

    # Build per-core input maps.
    in_maps = []
    for e in range(N_EXPERTS):
        tok = tok_lists[e]
        n = len(tok)
        xg = np.zeros((C, D_MODEL), dtype=np.float32)
        xg[:n] = x_flat[tok]
        wt = np.zeros((C,), dtype=np.float32)
        wt[:n] = np.where(idx[tok, 0] == e, w[tok, 0], w[tok, 1])

        # xT chunk-major: [p, ci, ko, c]
        xT_dev = np.ascontiguousarray(
            xg.reshape(NCHUNK, TB, KO1, P).transpose(3, 0, 2, 1)
        ).reshape(P, NCHUNK * KO1 * TB).astype(BF16)
        # w1 f-tile-major: [p, q, ftl, ko, f]
        w1_dev = np.ascontiguousarray(
            W1[e].reshape(KO1, P, NSLAB, FT_E, P).transpose(1, 2, 3, 0, 4)
        ).reshape(P, NSLAB * KO1 * FE).astype(BF16)
        w2_dev = np.ascontiguousarray(
            W2[e].reshape(KO2, P, D_MODEL).transpose(1, 0, 2)
        ).reshape(P, KO2 * D_MODEL).astype(BF16)
        b1_dev = np.ascontiguousarray(b1[e].reshape(FT, P).T)
        cw_dev = np.ascontiguousarray(wt.reshape(OUTERS, P).T)

        in_maps.append({
            "xT": xT_dev,
            "w1": w1_dev,
            "w2": w2_dev,
            "b1": b1_dev.astype(np.float32),
            "cw": cw_dev.astype(np.float32),
        })

    from concourse.bass_utils import run_bass_kernel_spmd

    global LAST_IN_MAPS, LAST_C
    LAST_IN_MAPS = in_maps
    LAST_C = C

    trace = os.environ.get("MOE_KERNEL_TRACE", "0") == "1"
    res = run_bass_kernel_spmd(
        nc, in_maps, core_ids=list(range(N_CORES)),
        trace=trace, trace_cores=[0] if trace else None,
    )
    LAST_RESULTS = res

    # Unpack per-core outputs: y_dev [P, (C/P)*D] -> [C, D]
    Yall = np.empty((N_EXPERTS, C, D_MODEL), dtype=np.float32)
    for e in range(N_EXPERTS):
        y_dev = res.results[e]["y"]
        Yall[e] = (
            y_dev.reshape(P, OUTERS, D_MODEL)
            .transpose(1, 0, 2)
            .reshape(C, D_MODEL)
        )

    # Combine device contributions (already scaled by cw on device).
    out_flat = np.zeros((T, D_MODEL), dtype=np.float32)
    for k in range(TOP_K):
        kept = slot[:, k] >= 0
        out_flat[kept] += Yall[idx[kept, k], slot[kept, k]]

    # Host overflow pairs in fp32.
    for e in range(N_EXPERTS):
        sp = spill_lists[e]
        if len(sp) == 0:
            continue
        ys = np.maximum(x_flat[sp] @ W1[e] + b1[e], 0.0) @ W2[e]
        ws = np.where(idx[sp, 0] == e, w[sp, 0], w[sp, 1])
        out_flat[sp] += ws[:, None] * ys

    if np.any(b2):
        out_flat += w[:, 0:1] * b2[idx[:, 0]] + w[:, 1:2] * b2[idx[:, 1]]

    return out_flat.reshape(B, S, D_MODEL).astype(np.float32)
